# revision 23
# baseline (speedup 1.0000x reference)
# Fused conv3x3(same) + bias + tanh + x2 + stride-4 subsample, data-parallel
# over 8 NeuronCores.
#
# Math: out[b,oc,y,x] = 2*tanh(sum_{ic,ky,kx} w[oc,ic,ky,kx]*x[b,ic,4y+ky-1,4x+kx-1] + bias[oc])
# computed in fp16 like the reference. Since the spatial stride (4) exceeds the
# kernel size (3), every output pixel reads a disjoint 3x3x8 input patch, so the
# conv lowers exactly to a [72 -> 64] GEMM over 64*64 pixels per image. The host
# does the im2col rearrangement (pure data movement, fp16 cast is identical to
# the reference's .astype(float16)); each core runs the GEMM + bias + tanh for
# 4 of the 32 images. The trailing *2 and fp32 cast are exact in either order,
# so they are applied on the host after the fp16 tanh.
#
# Device kernel is hand-scheduled raw bacc (no Tile framework: avoids its
# multi-microsecond preamble/tail barriers). The pipeline works in half-images
# (2048 pixels): 4 N=512 matmuls packed two-deep in PSUM partitions (chunk
# 2q+t -> partitions t*64:(t+1)*64 of bank q) so one 128-partition ACT computes
# tanh per half and the output DMA engages all SBUF ports. Output DRAM layout
# is [B, 2, 64, 2048] (t = chunk parity); the host interleaves it back.
#
# The contraction is zero-padded 72 -> 80 rows: an 80-partition DMA spreads
# over all 16 SDMA engines (a 72-partition one only gets 12), which is worth
# more than the 11% extra bytes — the kernel is input-DMA-stream-bound.
# Per-descriptor runs are kept at 4 KiB (~17 GB/s per engine vs ~14 at 8 KiB).
import sys

import numpy as np

try:
    import concourse.bass as bass  # noqa: F401
except ImportError:
    sys.path.insert(0, "/opt/trn_rl_repo")

import concourse.bass as bass  # noqa: F401
import concourse.bacc as bacc
import concourse.mybir as mybir
from concourse.bass_utils import run_bass_kernel_spmd

N_CORES = 8
B_FULL = 32
B_CORE = B_FULL // N_CORES  # 4 images per core
C_IN = 8
KH = KW = 3
K = C_IN * KH * KW  # 72 contraction
KP = 80  # zero-padded contraction (16-SDMA-engine alignment)
OC = 64
OH = OW = 64
NPIX = OH * OW  # 4096
HALF = NPIX // 2  # 2048
NH = 2 * B_CORE  # 8 half-image pipeline stages
F16 = mybir.dt.float16
F32 = mybir.dt.float32

_PROGRAM = None


def build_program():
    from contextlib import ExitStack

    nc = bacc.Bacc("TRN2")
    xp = nc.dram_tensor("xp", [B_CORE, KP, 2, HALF], F16, kind="ExternalInput")
    w = nc.dram_tensor("w", [KP, OC], F16, kind="ExternalInput")
    y = nc.dram_tensor("y", [NH, 2 * OC, HALF // 2], F16, kind="ExternalOutput")

    with ExitStack() as stack:
        w_tile = stack.enter_context(nc.sbuf_tensor([KP, OC], F16))
        # one buffer per half-image stage -> no buffer-reuse waits; each DMA
        # writes one contiguous 4KiB run per partition
        x_bufs = stack.enter_context(nc.sbuf_tensor([KP, NH, 2, HALF // 2 + 16], F16))
        a_bufs = stack.enter_context(nc.sbuf_tensor([2 * OC, NH, HALF // 2], F16))
        warm = stack.enter_context(nc.sbuf_tensor([2 * OC, 2 * OC], F16))
        # 8 banks of [128, 512]; stage i accumulates into banks 2i%8, 2i%8+1
        ps = stack.enter_context(nc.psum_tensor([2 * OC, 8, 512], F32))
        # Per-stage input semaphores: concurrent DMAs complete out of order,
        # so one counting sem can't tell which transfer landed. s_y only
        # gates the final all-done wait, where order doesn't matter.
        sx = [stack.enter_context(nc.semaphore(f"s_x{i}")) for i in range(NH)]
        s_w = stack.enter_context(nc.semaphore("s_w"))
        s_warm = stack.enter_context(nc.semaphore("s_warm"))
        s_mm = stack.enter_context(nc.semaphore("s_mm"))
        s_act = stack.enter_context(nc.semaphore("s_act"))
        s_y = stack.enter_context(nc.semaphore("s_y"))
        block = stack.enter_context(nc.Block())

        @block.gpsimd
        def _(gpsimd):
            gpsimd.memset(warm[:], 0.0).then_inc(s_warm, 1)

        @block.sync
        def _(sync):
            # first half-image heads the critical path; w is tiny. The bias
            # rides in w row K (patch row K is constant 1.0), so there is no
            # separate bias operand anywhere.
            sync.dma_start(out=x_bufs[:, 0, :, :1024], in_=xp[0][:, 0, :].rearrange('k (h n) -> k h n', h=2)).then_inc(sx[0], 16)
            sync.dma_start(out=w_tile[:], in_=w[:]).then_inc(s_w, 16)
            for i in range(1, NH):
                sync.dma_start(
                    out=x_bufs[:, i, :, :1024],
                    in_=xp[i // 2][:, i % 2, :].rearrange("k (h n) -> k h n", h=2),
                ).then_inc(sx[i], 16)
            # output stores, paced by the ACT chain; the scalar queue must
            # not carry them (a trigger costs ~0.6us and would serialize
            # with the 1.1us ACTs)
            for i in range(NH):
                sync.wait_ge(s_act, i + 1)
                sync.dma_start(out=y[i], in_=a_bufs[:, i]).then_inc(s_y, 16)
            sync.wait_ge(s_y, 16 * NH)

        @block.tensor
        def _(tensor):
            # keep the PE busy while inputs stream in so the HAM clock gate
            # opens (cold MMs run at 1.2GHz, warm at 2.4GHz); results land in
            # bank 7 which is overwritten by stage 3 later (start=True)
            tensor.wait_ge(s_warm, 1)
            for _ in range(50):
                nc.tensor.matmul(
                    ps[:OC, 7, :128],
                    warm[:, :OC],
                    warm[:],
                    start=True,
                    stop=True,
                )
            for i in range(NH):
                if i == 0:
                    tensor.wait_ge(s_w, 16)
                if i >= 4:
                    # psum bank pair reused; wait until ACT of stage i-4 read
                    # it. Taken BEFORE the input wait so the fillers below may
                    # touch this stage's banks.
                    tensor.wait_ge(s_act, i - 3)
                    # fillers: keep the PE busy while waiting for this
                    # stage's input so the HAM clock gate stays open (late
                    # stages otherwise re-throttle to 1.2GHz). They write
                    # this stage's own first bank, which the real start=True
                    # matmuls overwrite.
                    for _ in range(3):
                        nc.tensor.matmul(
                            ps[:OC, (2 * i) % 8, :128],
                            warm[:, :OC],
                            warm[:],
                            start=True,
                            stop=True,
                        )
                tensor.wait_ge(sx[i], 16)
                last = None
                for t in range(2):
                    for q in range(2):
                        c = 2 * q + t  # chunk within this half-image
                        last = nc.tensor.matmul(
                            ps[t * OC : (t + 1) * OC, (2 * i + q) % 8, :],
                            w_tile[:],
                            x_bufs[:, i, c // 2, (c % 2) * 512 : (c % 2 + 1) * 512],
                            start=True,
                            stop=True,
                        )
                last.then_inc(s_mm, 1)

        @block.scalar
        def _(scalar):
            for i in range(NH):
                scalar.wait_ge(s_mm, i + 1)
                bk = (2 * i) % 8
                nc.scalar.activation(
                    a_bufs[:, i],
                    ps[:, bk : bk + 2, :].rearrange("p b c -> p (b c)"),
                    mybir.ActivationFunctionType.Tanh,
                ).then_inc(s_act, 1)

    nc.finalize()
    return nc
